# revision 6
# baseline (speedup 1.0000x reference)
"""Trainium2 Bass kernel for nn_LinearStringEncoder (bag-of-words + Linear).

Math: out[i] = b + sum_{j < len_i} W[:, tokens[i,j]]  ==  hist_i @ W.T + b,
where hist_i is the token-count histogram of scene i over the vocab.

Strategy: instead of per-token descriptor gathers (SWDGE, ~1 us/desc), the
host packs the histogram as a dense fp8 matrix (counts are small ints, exact
in e4m3) and the device runs a streaming GEMM on the TensorEngine:

    out.T[h, s] = sum_v Wt[v, h] * histT[v, s]

Data-parallel over scenes: 512 scenes/core on 8 cores, no collectives.

Scene-tiled vocab compaction: scenes are sorted by length and grouped into
64-scene tiles; each tile only streams the ~6k vocab columns that actually
occur in it (W rows remapped to match, bias folded in as an extra vocab
row with hist = 1). That cuts per-core traffic from 38 MB (dense vocab) to
~14 MB. Per tile: lhsT = W chunk [128v, 128h] bf16 (stationary), rhs =
hist chunk [128v, 64s] fp8 (moving), chunks accumulate into the tile's own
PSUM bank [128h, 64s] f32; hist DMAs ride the SP HWDGE ring, W DMAs the
ACT ring so both descriptor queues drain in parallel.
"""

import sys

for _p in ("/opt/trn_rl_repo", "/root/.axon_site/_ro/trn_rl_repo"):
    if _p not in sys.path:
        sys.path.append(_p)

import ml_dtypes
import numpy as np

import concourse.bacc as bacc
import concourse.mybir as mybir
import concourse.tile as tile
from concourse.bass_utils import run_bass_kernel_spmd

B, L, V, H = 4096, 200, 50000, 128
NCORES = 8
SCENES = B // NCORES            # 512 scenes per core
ST = 64                         # scenes per tile
NT = SCENES // ST               # 8 tiles per core (one PSUM bank each)

F32 = mybir.dt.float32
BF16 = mybir.dt.bfloat16
FP8 = mybir.dt.float8e4

NP_FP8 = ml_dtypes.float8_e4m3
NP_BF16 = ml_dtypes.bfloat16


def _build_program(nchunks):
    """nchunks: tuple of NT per-tile vocab-chunk counts (same on all cores)."""
    totch = sum(nchunks)
    nc = bacc.Bacc("TRN2", debug=False, num_devices=NCORES)
    hist = nc.dram_tensor("hist", [128, totch, ST], FP8, kind="ExternalInput")
    wt = nc.dram_tensor("wt", [128, totch, H], BF16, kind="ExternalInput")
    out = nc.dram_tensor("out", [H, SCENES], F32, kind="ExternalOutput")

    with tile.TileContext(nc) as tc:
        with (
            tc.tile_pool(name="hp", bufs=3) as hp,
            tc.tile_pool(name="wp", bufs=3) as wp,
            tc.tile_pool(name="op", bufs=1) as op,
            tc.tile_pool(name="ps", bufs=8, space="PSUM") as ps,
        ):
            ot = op.tile([H, SCENES], F32)
            off = 0
            for t in range(NT):
                nck = nchunks[t]
                ht = hp.tile([128, nck, ST], FP8, tag="ht")
                wtt = wp.tile([128, nck, H], BF16, tag="wtt")
                nc.sync.dma_start(ht[:], hist[:, off:off + nck, :])
                # second HWDGE ring (ACT) so the two streams' descriptor
                # queues drain in parallel
                nc.scalar.dma_start(wtt[:], wt[:, off:off + nck, :])
                acc = ps.tile([H, ST], F32)
                for c in range(nck):
                    nc.tensor.matmul(
                        acc[:],
                        wtt[:, c, :],
                        ht[:, c, :],
                        start=(c == 0),
                        stop=(c == nck - 1),
                    )
                nc.vector.tensor_copy(out=ot[:, t * ST:(t + 1) * ST], in_=acc[:])
                off += nck
            nc.sync.dma_start(out[:], ot[:])
    nc.compile()
    return nc


_PROG_CACHE = {}


def _get_program(nchunks):
    if nchunks not in _PROG_CACHE:
        _PROG_CACHE[nchunks] = _build_program(nchunks)
    return _PROG_CACHE[nchunks]


# count -> fp8 byte lookup (counts are bounded by L=200 < 240 = e4m3 max)
_FP8_LUT = np.arange(256, dtype=np.float32).astype(NP_FP8)


def kernel(tokens, lengths, W, b):
    tokens = np.asarray(tokens).astype(np.int64)
    lengths = np.clip(np.asarray(lengths).astype(np.int64), 0, L)
    W = np.asarray(W, dtype=np.float32)
    b = np.asarray(b, dtype=np.float32)
    Wt = np.ascontiguousarray(W.T)                     # [V, H]

    # Sort scenes by length; global tile q = ranks [64q, 64q+64) -> core
    # q % 8, tile slot q // 8. Same-slot tiles then have similar token
    # counts on every core, so the shared per-slot chunk count is tight.
    order = np.argsort(-lengths, kind="stable")
    arangeL = np.arange(L)

    # Per global tile: unique cols, per-scene counts
    tiles = []                                         # [NT*NCORES] in q order
    for q in range(NT * NCORES):
        sc = order[q * ST:(q + 1) * ST]
        tok = tokens[sc]
        msk = arangeL[None, :] < lengths[sc][:, None]
        vals = tok[msk]
        sidx = np.broadcast_to(
            np.arange(ST, dtype=np.int64)[:, None], tok.shape)[msk]
        cols = np.unique(vals)
        tiles.append((sc, cols, vals, sidx))

    nchunks = tuple(
        max(-(-(len(tiles[t * NCORES + c][1]) + 1) // 128)
            for c in range(NCORES))
        for t in range(NT)
    )
    totch = sum(nchunks)

    in_maps = []
    for c in range(NCORES):
        hist_np = np.zeros((128, totch, ST), NP_FP8)
        wt_np = np.zeros((128, totch, H), NP_BF16)
        off = 0
        for t in range(NT):
            nck = nchunks[t]
            vp = nck * 128
            _, cols, vals, sidx = tiles[t * NCORES + c]
            u = len(cols)
            remap = np.searchsorted(cols, vals)
            cnt = np.bincount(sidx * vp + remap, minlength=ST * vp)
            cnt = cnt.astype(np.uint8).reshape(ST, vp)
            cnt[:, u] = 1                              # bias row
            hist_np[:, off:off + nck, :] = _FP8_LUT[
                cnt.reshape(ST, nck, 128).transpose(2, 1, 0)]

            w_small = np.zeros((vp, H), np.float32)
            w_small[:u] = Wt[cols]
            w_small[u] = b
            wt_np[:, off:off + nck, :] = w_small.reshape(
                nck, 128, H).transpose(1, 0, 2).astype(NP_BF16)
            off += nck
        in_maps.append({"hist": hist_np, "wt": wt_np})

    nc = _get_program(nchunks)
    res = run_bass_kernel_spmd(nc, in_maps, core_ids=list(range(NCORES)))

    out_full = np.empty((B, H), np.float32)
    for q in range(NT * NCORES):
        c, t = q % NCORES, q // NCORES
        sc = tiles[q][0]
        out_full[sc] = res.results[c]["out"][:, t * ST:(t + 1) * ST].T
    return out_full
